# revision 1
# baseline (speedup 1.0000x reference)
"""Trainium2 Bass kernel for ChannelAttention (B=16, C=512, H=W=64).

Math (per batch b):
    xf = x[b] reshaped [C, N], N = H*W = 4096
    q = Wq @ xf + bq            [64, N]
    k = Wk @ xf + bk            [64, N]
    v = Wv @ xf + bv            [64, N]
    energy = q @ k.T            [64, 64]   (contraction over N)
    attn = softmax(energy, -1)
    z = attn @ v                [64, N]
    out = Wo @ z + bo           [C, N]

Sharding: data-parallel over batch, 2 batches per core on 8 cores, no
collectives.  Each core receives its x shard plus the (host-pre-transposed)
weights and returns its out shard.

On-chip dataflow per batch (8 n-panels of 512), default scheme "b":
  - qT|kT projected DIRECTLY in transposed [n, q|k] layout: per 128-wide
    n-subtile, 4 accumulating matmuls with the xf c-chunk as the stationary
    operand (lhsT) and [WqT|WkT] as the moving operand.  This avoids any
    explicit transposes; biases are added along the free dim with a
    broadcast tile during the PSUM->SBUF copy on DVE.  The energy
    [64, 64] accumulates over all 32 n-subtiles as qT.T @ kT in full fp32
    (softmax is sensitive to energy error: values are ~N(0, 64^2), so the
    top-2 gap can be small and tf32-level error would be amplified).
  - v projected in native [64, n] layout (float32r, 1 cycle/row), kept in
    SBUF for the whole batch.
  - softmax: DVE row-max (negated), ACT exp with bias=-max and accum_out
    row-sum, DVE reciprocal + row scale of attn in place.
  - out = Wo @ (attn @ v) + bo is reassociated as (Wo @ attn) @ v:
    W2T = attn.T-free matmul (lhsT=attn native, rhs=WoT, one instruction),
    then out m-tiles = W2T-slice.T @ v panels in float32r, bias added on
    DVE during the PSUM->SBUF copy, DMA'd out per [128, 512] tile.

Matmul dtype notes: float32 is exact but runs at 4 cycles/row on the PE;
float32r runs at 1 cycle/row (for free dim >= 256) with ~tf32 multiply
precision (measured ~5e-4 relative on this problem's linear paths).
fp32r operands must be *typed* float32r at their producer (DMA from an
fp32r DRAM tensor, or an ACT/DVE copy with fp32r output); the bytes are
plain fp32 and can be bitcast back for exact fp32 consumers.
Measured on HW: rel L2 error 6.2e-4 vs the fp32 reference; ~150-180 us
per-core device time (2 batches/core), vs a ~70 us pure-DMA floor.
"""

import os

import numpy as np

# Problem shape (hardcoded; kernel.py must be self-contained).
B, C, H, W = 16, 512, 64, 64
N = H * W  # 4096
C8 = 64
P = 128
NCORES = 8
BPC = B // NCORES  # batches per core
CCH = C // P  # 4 c-chunks of 128
NP = 512  # n-panel width
NPANELS = N // NP  # 8
NSUB = NP // P  # 4 transpose subtiles per panel

# Matmul dtype knobs ("f32" = exact, "f32r" = fast single-pass).
QK_DT = os.environ.get("CHATT_QK_DT", "f32")
V_DT = os.environ.get("CHATT_V_DT", "f32r")
EN_DT = os.environ.get("CHATT_EN_DT", "f32")
ZO_DT = os.environ.get("CHATT_ZO_DT", "f32r")
# Timing aid: repeat the whole body REPS times inside a hardware loop so the
# device time is measurable above the host<->device transfer noise.
REPS = int(os.environ.get("CHATT_REPS", "1"))
# Bisection aids (timing experiments only; outputs become wrong):
SKIP_ENERGY = os.environ.get("CHATT_SKIP_ENERGY", "0") == "1"
SKIP_PHASEB = os.environ.get("CHATT_SKIP_PHASEB", "0") == "1"
# Energy-path structure:
#  "t": project q|k in native layout, PE-transpose panels, energy from qkT
#  "b": project qT|kT directly (xf chunks as stationary operand) - fewer
#       cross-engine hops, exact fp32 energy path, no transposes
SCHEME = os.environ.get("CHATT_SCHEME", "b")
# Engine for the out-tile PSUM->SBUF bias copies: "dve", "act", or "alt"
OUT_ENG = os.environ.get("CHATT_OUT_ENG", "dve")
# Out-DMA granularity: "mtile" = [128,512] per (mo,panel); "panel" = staged
# [512,512] per panel (fewer, bigger DMAs)
OUT_STAGE = os.environ.get("CHATT_OUT_STAGE", "mtile")
# Input DMA granularity: panels per dma_start (1 -> 1MB, 2 -> 2MB)
XF_PANELS = int(os.environ.get("CHATT_XF_PANELS", "1"))

_CACHE = {}
LAST_RESULTS = None


def _build_program():
    import concourse.bass as bass  # noqa: F401
    import concourse.mybir as mybir
    import concourse.tile as tile
    from concourse import bacc
    from concourse.masks import make_identity
    from contextlib import ExitStack

    f32 = mybir.dt.float32
    f32r = mybir.dt.float32r

    def dt_of(kind):
        return f32r if kind == "f32r" else f32

    # xf feeds both the qk and v projections; it is typed f32r if either
    # consumer is f32r, and bitcast back to f32 for an exact consumer
    # (fp32r bytes are fp32 bytes; the precision reduction happens in the PE).
    xf_dt = f32r if (QK_DT == "f32r" or V_DT == "f32r") else f32

    def x_cast(ap, kind):
        # cast xf slice to the dtype wanted by this matmul
        want = dt_of(kind)
        return ap if ap.dtype == want else ap.bitcast(want)

    nc = bacc.Bacc("TRN2", target_bir_lowering=False)

    x_h = nc.dram_tensor("x", [BPC, C, N], xf_dt, kind="ExternalInput")
    wqk_h = nc.dram_tensor("w_qkt", [C, P], dt_of(QK_DT), kind="ExternalInput")
    wv_h = nc.dram_tensor("w_vt", [C, C8], dt_of(V_DT), kind="ExternalInput")
    wo_h = nc.dram_tensor("w_ot", [C8, C], dt_of(ZO_DT), kind="ExternalInput")
    bqk_h = nc.dram_tensor("b_qk", [P], f32, kind="ExternalInput")
    bv_h = nc.dram_tensor("b_v", [C8], f32, kind="ExternalInput")
    bo_h = nc.dram_tensor("b_o", [C], f32, kind="ExternalInput")
    y_h = nc.dram_tensor("y", [BPC, C, N], f32, kind="ExternalOutput")

    x_ap = x_h.ap()
    y_ap = y_h.ap()

    with tile.TileContext(nc) as tc, ExitStack() as ctx:
        def _n(name, default):
            return int(os.environ.get(f"CHATT_BUFS_{name}", str(default)))

        consts = ctx.enter_context(tc.tile_pool(name="consts", bufs=1))
        xp = ctx.enter_context(
            tc.tile_pool(name="xp", bufs=_n("XP", max(2, 8 // XF_PANELS)))
        )
        qkp = ctx.enter_context(tc.tile_pool(name="qkp", bufs=_n("QKP", 3)))
        qktp = ctx.enter_context(tc.tile_pool(name="qktp", bufs=_n("QKTP", 4)))
        vp = ctx.enter_context(tc.tile_pool(name="vp", bufs=2))
        zp = ctx.enter_context(tc.tile_pool(name="zp", bufs=3))
        op = ctx.enter_context(
            tc.tile_pool(name="op", bufs=_n("OP", 6 if OUT_STAGE == "mtile" else 3))
        )
        smallp = ctx.enter_context(tc.tile_pool(name="smallp", bufs=4))
        # PSUM: 8 banks total.
        # scheme t: proj(qk+v) 3 + transpose 2 + energy 1 + out 2
        # scheme b: proj(v) 2 + qkT 3 + energy 1 + out 2
        ps_cfg = os.environ.get("CHATT_PSUM", "b" if SCHEME == "b" else "a")
        pe_n = 1
        if ps_cfg == "b":
            pp_n, pt_n, pzo_n = (2, 3, 2)
        elif ps_cfg == "e2":
            # double-buffer the energy bank so batch b+1's energy
            # accumulation doesn't wait for batch b's softmax
            pp_n, pt_n, pe_n, pzo_n = 2, 2, 2, 2
        else:
            pp_n, pt_n, pzo_n = (3, 2, 2)
        pp = ctx.enter_context(tc.tile_pool(name="pp", bufs=pp_n, space="PSUM"))
        pt = ctx.enter_context(tc.tile_pool(name="pt", bufs=pt_n, space="PSUM"))
        pe = ctx.enter_context(tc.tile_pool(name="pe", bufs=pe_n, space="PSUM"))
        pzo = ctx.enter_context(tc.tile_pool(name="pzo", bufs=pzo_n, space="PSUM"))

        # One-time constants.
        wqk_sb = consts.tile([P, CCH, P], dt_of(QK_DT))
        nc.sync.dma_start(wqk_sb, wqk_h.ap().rearrange("(co ci) m -> ci co m", ci=P))
        wv_sb = consts.tile([P, CCH, C8], dt_of(V_DT))
        nc.sync.dma_start(wv_sb, wv_h.ap().rearrange("(co ci) m -> ci co m", ci=P))
        wo_sb = consts.tile([C8, C], dt_of(ZO_DT))
        nc.sync.dma_start(wo_sb, wo_h.ap())
        bqk_sb = consts.tile([P, 1], f32)
        nc.sync.dma_start(bqk_sb, bqk_h.ap()[:, None])
        bv_sb = consts.tile([C8, 1], f32)
        nc.sync.dma_start(bv_sb, bv_h.ap()[:, None])
        bo_sb = consts.tile([P, CCH], f32)
        nc.sync.dma_start(bo_sb, bo_h.ap().rearrange("(mo mi) -> mi mo", mi=P))
        ident = consts.tile([P, P], f32)
        make_identity(nc, ident)
        if SCHEME == "b":
            # b_qk broadcast to all partitions: [128, 128] with the bias
            # along the free dim (for the transposed-layout bias add)
            bqk_bc = consts.tile([P, P], f32)
            nc.sync.dma_start(
                bqk_bc,
                bass.AP(tensor=bqk_h, offset=0, ap=[[0, P], [1, P]]),
            )

        Identity = mybir.ActivationFunctionType.Identity
        Copy = mybir.ActivationFunctionType.Copy
        Exp = mybir.ActivationFunctionType.Exp

        from contextlib import nullcontext

        hint = (
            (
                mybir.EngineType.PE,
                mybir.EngineType.Activation,
                mybir.EngineType.DVE,
                mybir.EngineType.SP,
            )
            if os.environ.get("CHATT_HINT", "0") == "1"
            else ()
        )
        rep_cm = (
            tc.For_i(0, REPS, 1, hint_engines=hint) if REPS > 1 else nullcontext()
        )
        with rep_cm:
            for b in range(BPC):
                xb = x_ap[b].rearrange("(co ci) n -> ci co n", ci=P)
                yb = y_ap[b].rearrange("(mo mi) n -> mi mo n", mi=P)

                energy = pe.tile([C8, C8], f32, tag="energy", name=f"energy_{b}")
                v_sb = vp.tile([C8, N], dt_of(ZO_DT), tag="v", name=f"v_{b}")

                # ---- Phase A: projections + energy accumulation ----
                xf_group = {}
                for p in range(NPANELS):
                    nsl = slice(p * NP, (p + 1) * NP)
                    if p % XF_PANELS == 0:
                        gw = XF_PANELS * NP
                        xf_g = xp.tile(
                            [P, CCH, gw], xf_dt, tag="xf", name=f"xf_{b}_{p}"
                        )
                        nc.sync.dma_start(
                            xf_g, xb[:, :, p * NP : p * NP + gw]
                        )
                        xf_group = {"tile": xf_g, "base": p}
                    off = (p - xf_group["base"]) * NP
                    xf = xf_group["tile"][:, :, off : off + NP]

                    v_ps = pp.tile([C8, NP], f32, tag="proj", name=f"vps_{b}_{p}")
                    for co in range(CCH):
                        nc.tensor.matmul(
                            v_ps,
                            wv_sb[:, co, :],
                            x_cast(xf[:, co, :], V_DT),
                            start=(co == 0),
                            stop=(co == CCH - 1),
                        )
                    nc.scalar.activation(
                        v_sb[:, nsl], v_ps, Identity, bias=bv_sb, scale=1.0
                    )

                    last_p = 0 if SKIP_ENERGY else NPANELS - 1
                    if SCHEME == "b":
                        if not (SKIP_ENERGY and p > 0):
                            for ns in range(NSUB):
                                qt_ps = pt.tile(
                                    [P, P], f32, tag="tp", name=f"qtps_{b}_{p}_{ns}"
                                )
                                for co in range(CCH):
                                    nc.tensor.matmul(
                                        qt_ps,
                                        x_cast(
                                            xf[:, co, ns * P : (ns + 1) * P], QK_DT
                                        ),
                                        wqk_sb[:, co, :],
                                        start=(co == 0),
                                        stop=(co == CCH - 1),
                                    )
                                qkt_sb = qktp.tile(
                                    [P, P],
                                    dt_of(EN_DT),
                                    tag="qkt",
                                    name=f"qkt_{b}_{p}_{ns}",
                                )
                                nc.vector.tensor_tensor(
                                    qkt_sb, qt_ps, bqk_bc, mybir.AluOpType.add
                                )
                                nc.tensor.matmul(
                                    energy,
                                    qkt_sb[:, 0:C8],
                                    qkt_sb[:, C8:P],
                                    start=(p == 0 and ns == 0),
                                    stop=(p == last_p and ns == NSUB - 1),
                                )
                    else:
                        qk_ps = pp.tile([P, NP], f32, tag="proj", name=f"qkps_{b}_{p}")
                        for co in range(CCH):
                            nc.tensor.matmul(
                                qk_ps,
                                wqk_sb[:, co, :],
                                x_cast(xf[:, co, :], QK_DT),
                                start=(co == 0),
                                stop=(co == CCH - 1),
                            )
                        qk_sb = qkp.tile([P, NP], f32, tag="qk", name=f"qk_{b}_{p}")
                        nc.scalar.activation(
                            qk_sb, qk_ps, Identity, bias=bqk_sb, scale=1.0
                        )
                        if not (SKIP_ENERGY and p > 0):
                            for ns in range(NSUB):
                                t_ps = pt.tile(
                                    [P, P], f32, tag="tp", name=f"tps_{b}_{p}_{ns}"
                                )
                                nc.tensor.transpose(
                                    t_ps, qk_sb[:, ns * P : (ns + 1) * P], ident
                                )
                                qkt_sb = qktp.tile(
                                    [P, P],
                                    dt_of(EN_DT),
                                    tag="qkt",
                                    name=f"qkt_{b}_{p}_{ns}",
                                )
                                nc.vector.tensor_copy(qkt_sb, t_ps)
                                nc.tensor.matmul(
                                    energy,
                                    qkt_sb[:, 0:C8],
                                    qkt_sb[:, C8:P],
                                    start=(p == 0 and ns == 0),
                                    stop=(p == last_p and ns == NSUB - 1),
                                )

                # ---- Phase B: softmax, W2 = Wo @ (attn/rowsum), out = W2 @ v
                negmax = smallp.tile([C8, 1], f32, tag="negmax", name=f"negmax_{b}")
                nc.vector.reduce_max(
                    negmax, energy, axis=mybir.AxisListType.X, negate=True
                )
                attn = smallp.tile([C8, C8], f32, tag="attn", name=f"attn_{b}")
                rowsum = smallp.tile([C8, 1], f32, tag="rowsum", name=f"rowsum_{b}")
                nc.scalar.activation(
                    attn, energy, Exp, bias=negmax, scale=1.0, accum_out=rowsum
                )
                recip = smallp.tile([C8, 1], f32, tag="recip", name=f"recip_{b}")
                nc.vector.reciprocal(recip, rowsum)
                # normalize attn rows in place (per-partition scale)
                nc.vector.tensor_scalar_mul(attn, attn, recip)

                # W2T[d, o] = sum_c attn[c, d] WoT[c, o]  (one matmul)
                w2_ps = pt.tile([C8, C], f32, tag="tp", name=f"w2ps_{b}")
                nc.tensor.matmul(w2_ps, attn, wo_sb.bitcast(f32), start=True, stop=True)
                w2_sb = zp.tile([C8, C], dt_of(ZO_DT), tag="z", name=f"w2_{b}")
                nc.vector.tensor_copy(w2_sb, w2_ps)

                if SKIP_PHASEB:
                    for p in range(NPANELS):
                        nsl = slice(p * NP, (p + 1) * NP)
                        nc.sync.dma_start(yb[:C8, 0, nsl], v_sb[:, nsl].bitcast(f32))
                    continue
                if OUT_STAGE == "panel":
                    for p in range(NPANELS):
                        nsl = slice(p * NP, (p + 1) * NP)
                        o_sb = op.tile(
                            [P, CCH, NP], f32, tag="o", name=f"o_{b}_{p}"
                        )
                        for mo in range(CCH):
                            o_ps = pzo.tile(
                                [P, NP], f32, tag="zo", name=f"ops_{b}_{p}_{mo}"
                            )
                            nc.tensor.matmul(
                                o_ps,
                                w2_sb[:, mo * P : (mo + 1) * P],
                                v_sb[:, nsl],
                                start=True,
                                stop=True,
                            )
                            use_act = OUT_ENG == "act" or (
                                OUT_ENG == "alt" and mo % 2 == 1
                            )
                            if use_act:
                                nc.scalar.activation(
                                    o_sb[:, mo, :], o_ps, Identity,
                                    bias=bo_sb[:, mo : mo + 1], scale=1.0,
                                )
                            else:
                                nc.vector.tensor_scalar_add(
                                    o_sb[:, mo, :], o_ps, bo_sb[:, mo : mo + 1]
                                )
                        nc.sync.dma_start(yb[:, :, nsl], o_sb)
                else:
                    for mo in range(CCH):
                        for p in range(NPANELS):
                            nsl = slice(p * NP, (p + 1) * NP)
                            o_ps = pzo.tile(
                                [P, NP], f32, tag="zo", name=f"ops_{b}_{p}_{mo}"
                            )
                            nc.tensor.matmul(
                                o_ps,
                                w2_sb[:, mo * P : (mo + 1) * P],
                                v_sb[:, nsl],
                                start=True,
                                stop=True,
                            )
                            o_sb = op.tile(
                                [P, NP], f32, tag="o", name=f"o_{b}_{p}_{mo}"
                            )
                            use_act = OUT_ENG == "act" or (
                                OUT_ENG == "alt" and p % 2 == 1
                            )
                            if use_act:
                                nc.scalar.activation(
                                    o_sb, o_ps, Identity,
                                    bias=bo_sb[:, mo : mo + 1], scale=1.0,
                                )
                            else:
                                nc.vector.tensor_scalar_add(
                                    o_sb, o_ps, bo_sb[:, mo : mo + 1]
                                )
                            nc.sync.dma_start(yb[:, mo, nsl], o_sb)

    nc.compile()
    return nc


def _get_program():
    key = (QK_DT, V_DT, EN_DT, ZO_DT, REPS)
    if key not in _CACHE:
        _CACHE[key] = _build_program()
    return _CACHE[key]


def _host_inputs(x, Wq, bq, Wk, bk, Wv, bv, Wo, bo):
    """Build the per-core input maps (host-side shard + weight transposes)."""
    x = np.ascontiguousarray(x, dtype=np.float32).reshape(B, C, N)
    w_qkt = np.ascontiguousarray(
        np.concatenate([Wq, Wk], axis=0).T.astype(np.float32)
    )  # [C, 128]
    w_vt = np.ascontiguousarray(Wv.T.astype(np.float32))  # [C, 64]
    w_ot = np.ascontiguousarray(Wo.T.astype(np.float32))  # [64, C]
    b_qk = np.ascontiguousarray(
        np.concatenate([bq, bk], axis=0).astype(np.float32)
    )  # [128]
    b_v = np.ascontiguousarray(bv.astype(np.float32))
    b_o = np.ascontiguousarray(bo.astype(np.float32))

    in_maps = []
    for i in range(NCORES):
        in_maps.append(
            {
                "x": np.ascontiguousarray(x[i * BPC : (i + 1) * BPC]),
                "w_qkt": w_qkt,
                "w_vt": w_vt,
                "w_ot": w_ot,
                "b_qk": b_qk,
                "b_v": b_v,
                "b_o": b_o,
            }
        )
    return in_maps


def kernel(**inputs):
    global LAST_RESULTS
    from concourse.bass_utils import run_bass_kernel_spmd

    nc = _get_program()
    in_maps = _host_inputs(**inputs)
    res = run_bass_kernel_spmd(nc, in_maps, core_ids=list(range(NCORES)))
    LAST_RESULTS = res
    out = np.concatenate([r["y"] for r in res.results], axis=0)
    return out.reshape(B, C, H, W).astype(np.float32)



# revision 15
# speedup vs baseline: 1.3891x; 1.3891x over previous
"""Trainium2 Bass kernel for ChannelAttention (B=16, C=512, H=W=64).

Math (per batch b):
    xf = x[b] reshaped [C, N], N = H*W = 4096
    q = Wq @ xf + bq            [64, N]
    k = Wk @ xf + bk            [64, N]
    v = Wv @ xf + bv            [64, N]
    energy = q @ k.T            [64, 64]   (contraction over N)
    attn = softmax(energy, -1)
    z = attn @ v                [64, N]
    out = Wo @ z + bo           [C, N]

Sharding: data-parallel over batch, 2 batches per core on 8 cores, no
collectives.  Each core receives its x shard plus the (host-pre-transposed)
weights and returns its out shard.

On-chip dataflow per batch (8 n-panels of 512), default scheme "b":
  - qT|kT projected DIRECTLY in transposed [n, q|k] layout: per 128-wide
    n-subtile, 4 accumulating matmuls with the xf c-chunk as the stationary
    operand (lhsT) and [WqT|WkT] as the moving operand.  This avoids any
    explicit transposes; biases are added along the free dim with a
    broadcast tile during the PSUM->SBUF copy on DVE.  The energy
    [64, 64] accumulates over all 32 n-subtiles as qT.T @ kT in full fp32
    (softmax is sensitive to energy error: values are ~N(0, 64^2), so the
    top-2 gap can be small and tf32-level error would be amplified).
  - v projected in native [64, n] layout (float32r, 1 cycle/row), kept in
    SBUF for the whole batch.
  - softmax: DVE row-max (negated), ACT exp with bias=-max and accum_out
    row-sum, DVE reciprocal + row scale of attn in place.
  - out = Wo @ (attn @ v) + bo is reassociated as (Wo @ attn) @ v:
    W2T = attn.T-free matmul (lhsT=attn native, rhs=WoT, one instruction),
    then out m-tiles = W2T-slice.T @ v panels in float32r, bias added on
    DVE during the PSUM->SBUF copy, DMA'd out per [128, 512] tile.

Matmul dtype notes: float32 is exact but runs at 4 cycles/row on the PE;
float32r runs at 1 cycle/row (for free dim >= 256) with ~tf32 multiply
precision (measured ~5e-4 relative on this problem's linear paths).
fp32r operands must be *typed* float32r at their producer (DMA from an
fp32r DRAM tensor, or an ACT/DVE copy with fp32r output); the bytes are
plain fp32 and can be bitcast back for exact fp32 consumers.
Measured on HW: rel L2 error 6.2e-4 vs the fp32 reference; ~150-180 us
per-core device time (2 batches/core), vs a ~70 us pure-DMA floor.
"""

import os

import numpy as np

# Problem shape (hardcoded; kernel.py must be self-contained).
B, C, H, W = 16, 512, 64, 64
N = H * W  # 4096
C8 = 64
P = 128
NCORES = 8
BPC = B // NCORES  # batches per core
CCH = C // P  # 4 c-chunks of 128
NP = 512  # n-panel width
NPANELS = N // NP  # 8
NSUB = NP // P  # 4 transpose subtiles per panel

# Matmul dtype knobs ("f32" = exact, "f32r" = fast single-pass).
QK_DT = os.environ.get("CHATT_QK_DT", "f32r")
V_DT = os.environ.get("CHATT_V_DT", "f32r")
EN_DT = os.environ.get("CHATT_EN_DT", "f32")
ZO_DT = os.environ.get("CHATT_ZO_DT", "f32r")
# Output DRAM dtype: bf16 halves the store traffic (host converts back to
# f32); rel-L2 impact ~1e-3, well under the 2e-2 gate.
Y_DT = os.environ.get("CHATT_Y_DT", "bf16")
# Timing aid: repeat the whole body REPS times inside a hardware loop so the
# device time is measurable above the host<->device transfer noise.
REPS = int(os.environ.get("CHATT_REPS", "1"))
# Bisection aids (timing experiments only; outputs become wrong):
SKIP_ENERGY = os.environ.get("CHATT_SKIP_ENERGY", "0") == "1"
SKIP_PHASEB = os.environ.get("CHATT_SKIP_PHASEB", "0") == "1"
# Energy-path structure:
#  "t": project q|k in native layout, PE-transpose panels, energy from qkT
#      (with QK_DT=f32r the projection runs 1 cycle/row: free dim 512)
#  "b": project qT|kT directly (xf chunks as stationary operand) - fewer
#       cross-engine hops, exact fp32 energy path, no transposes, but the
#       projection free dim is 128 so fp32/fp32r both run 4 cycles/row
SCHEME = os.environ.get("CHATT_SCHEME", "t")
# Engine for the out-tile PSUM->SBUF bias copies: "dve", "act", or "alt"
OUT_ENG = os.environ.get("CHATT_OUT_ENG", "dve")
# Out-DMA granularity: "mtile" = [128,512] per (mo,panel); "panel" = staged
# [512,512] per panel (fewer, bigger DMAs)
OUT_STAGE = os.environ.get("CHATT_OUT_STAGE", "mtile")
# Input DMA granularity: panels per dma_start (1 -> 1MB, 2 -> 2MB)
XF_PANELS = int(os.environ.get("CHATT_XF_PANELS", "2"))

_CACHE = {}
LAST_RESULTS = None


def _build_program():
    import concourse.bass as bass  # noqa: F401
    import concourse.mybir as mybir
    import concourse.tile as tile
    from concourse import bacc
    from concourse.masks import make_identity
    from contextlib import ExitStack

    f32 = mybir.dt.float32
    f32r = mybir.dt.float32r
    bf16 = mybir.dt.bfloat16
    y_dt = bf16 if Y_DT == "bf16" else f32

    def dt_of(kind):
        if kind == "bf16":
            return bf16
        return f32r if kind == "f32r" else f32

    # xf feeds both the qk and v projections; it is typed f32r if either
    # consumer is f32r, and bitcast back to f32 for an exact consumer
    # (fp32r bytes are fp32 bytes; the precision reduction happens in the PE).
    xf_dt = f32r if (QK_DT == "f32r" or V_DT == "f32r") else f32

    def x_cast(ap, kind):
        # cast xf slice to the dtype wanted by this matmul
        want = dt_of(kind)
        return ap if ap.dtype == want else ap.bitcast(want)

    nc = bacc.Bacc("TRN2", target_bir_lowering=False)

    x_h = nc.dram_tensor("x", [BPC, C, N], xf_dt, kind="ExternalInput")
    wqk_h = nc.dram_tensor("w_qkt", [C, P], dt_of(QK_DT), kind="ExternalInput")
    wv_h = nc.dram_tensor("w_vt", [C, C8], dt_of(V_DT), kind="ExternalInput")
    wo_h = nc.dram_tensor("w_ot", [C8, C], dt_of(ZO_DT), kind="ExternalInput")
    bqk_h = nc.dram_tensor("b_qk", [P], f32, kind="ExternalInput")
    bv_h = nc.dram_tensor("b_v", [C8], f32, kind="ExternalInput")
    bo_h = nc.dram_tensor("b_o", [C], f32, kind="ExternalInput")
    y_h = nc.dram_tensor("y", [BPC, C, N], y_dt, kind="ExternalOutput")

    x_ap = x_h.ap()
    y_ap = y_h.ap()

    with tile.TileContext(nc) as tc, ExitStack() as ctx:
        def _n(name, default):
            return int(os.environ.get(f"CHATT_BUFS_{name}", str(default)))

        consts = ctx.enter_context(tc.tile_pool(name="consts", bufs=1))
        xp = ctx.enter_context(
            tc.tile_pool(name="xp", bufs=_n("XP", max(2, 8 // XF_PANELS)))
        )
        qkp = ctx.enter_context(tc.tile_pool(name="qkp", bufs=_n("QKP", 3)))
        qktp = ctx.enter_context(tc.tile_pool(name="qktp", bufs=_n("QKTP", 4)))
        vp = ctx.enter_context(tc.tile_pool(name="vp", bufs=2))
        zp = ctx.enter_context(tc.tile_pool(name="zp", bufs=3))
        op = ctx.enter_context(
            tc.tile_pool(name="op", bufs=_n("OP", 6 if OUT_STAGE == "mtile" else 3))
        )
        smallp = ctx.enter_context(tc.tile_pool(name="smallp", bufs=4))
        # PSUM: 8 banks total.
        # scheme t: proj(qk+v) 3 + transpose 2 + energy 1 + out 2
        # scheme b: proj(v) 2 + qkT 3 + energy 1 + out 2
        ps_cfg = os.environ.get("CHATT_PSUM", "b" if SCHEME == "b" else "a")
        pe_n = 1
        if ps_cfg == "b":
            pp_n, pt_n, pzo_n = (2, 3, 2)
        elif ps_cfg == "e2":
            # double-buffer the energy bank so batch b+1's energy
            # accumulation doesn't wait for batch b's softmax
            pp_n, pt_n, pe_n, pzo_n = 2, 2, 2, 2
        else:
            pp_n, pt_n, pzo_n = (3, 2, 2)
        pp = ctx.enter_context(tc.tile_pool(name="pp", bufs=pp_n, space="PSUM"))
        pt = ctx.enter_context(tc.tile_pool(name="pt", bufs=pt_n, space="PSUM"))
        pe = ctx.enter_context(tc.tile_pool(name="pe", bufs=pe_n, space="PSUM"))
        pzo = ctx.enter_context(tc.tile_pool(name="pzo", bufs=pzo_n, space="PSUM"))

        # One-time constants.
        wqk_sb = consts.tile([P, CCH, P], dt_of(QK_DT))
        nc.sync.dma_start(wqk_sb, wqk_h.ap().rearrange("(co ci) m -> ci co m", ci=P))
        wv_sb = consts.tile([P, CCH, C8], dt_of(V_DT))
        nc.sync.dma_start(wv_sb, wv_h.ap().rearrange("(co ci) m -> ci co m", ci=P))
        wo_sb = consts.tile([C8, C], dt_of(ZO_DT))
        nc.sync.dma_start(wo_sb, wo_h.ap())
        bqk_sb = consts.tile([P, 1], f32)
        nc.sync.dma_start(bqk_sb, bqk_h.ap()[:, None])
        bv_sb = consts.tile([C8, 1], f32)
        nc.sync.dma_start(bv_sb, bv_h.ap()[:, None])
        bo_sb = consts.tile([P, CCH], f32)
        nc.sync.dma_start(bo_sb, bo_h.ap().rearrange("(mo mi) -> mi mo", mi=P))
        ident = consts.tile([P, P], f32)
        make_identity(nc, ident)
        ident_r = None
        if SCHEME == "t" and dt_of(QK_DT) != f32:
            # fp32r operands must be *typed* fp32r at their producer; a
            # bitcast of the f32 identity is rejected by the BIR verifier,
            # and memset can't write f32r, so copy through DVE.
            ident_r = consts.tile([P, P], dt_of(QK_DT))
            nc.vector.tensor_copy(ident_r, ident)
        if SCHEME == "b":
            # b_qk broadcast to all partitions: [128, 128] with the bias
            # along the free dim (for the transposed-layout bias add)
            bqk_bc = consts.tile([P, P], f32)
            nc.sync.dma_start(
                bqk_bc,
                bass.AP(tensor=bqk_h, offset=0, ap=[[0, P], [1, P]]),
            )

        Identity = mybir.ActivationFunctionType.Identity
        Copy = mybir.ActivationFunctionType.Copy
        Exp = mybir.ActivationFunctionType.Exp

        from contextlib import nullcontext

        hint = (
            (
                mybir.EngineType.PE,
                mybir.EngineType.Activation,
                mybir.EngineType.DVE,
                mybir.EngineType.SP,
            )
            if os.environ.get("CHATT_HINT", "0") == "1"
            else ()
        )
        rep_cm = (
            tc.For_i(0, REPS, 1, hint_engines=hint) if REPS > 1 else nullcontext()
        )
        with rep_cm:
            for b in range(BPC):
                xb = x_ap[b].rearrange("(co ci) n -> ci co n", ci=P)
                yb = y_ap[b].rearrange("(mo mi) n -> mi mo n", mi=P)

                energy = pe.tile([C8, C8], f32, tag="energy", name=f"energy_{b}")
                v_sb = vp.tile([C8, N], dt_of(ZO_DT), tag="v", name=f"v_{b}")

                # ---- Phase A: projections + energy accumulation ----
                xf_group = {}
                for p in range(NPANELS):
                    nsl = slice(p * NP, (p + 1) * NP)
                    if p % XF_PANELS == 0:
                        gw = XF_PANELS * NP
                        xf_g = xp.tile(
                            [P, CCH, gw], xf_dt, tag="xf", name=f"xf_{b}_{p}"
                        )
                        nc.sync.dma_start(
                            xf_g, xb[:, :, p * NP : p * NP + gw]
                        )
                        xf_group = {"tile": xf_g, "base": p}
                    off = (p - xf_group["base"]) * NP
                    xf = xf_group["tile"][:, :, off : off + NP]

                    v_ps = pp.tile([C8, NP], f32, tag="proj", name=f"vps_{b}_{p}")
                    for co in range(CCH):
                        nc.tensor.matmul(
                            v_ps,
                            wv_sb[:, co, :],
                            x_cast(xf[:, co, :], V_DT),
                            start=(co == 0),
                            stop=(co == CCH - 1),
                        )
                    nc.scalar.activation(
                        v_sb[:, nsl], v_ps, Identity, bias=bv_sb, scale=1.0
                    )

                    last_p = 0 if SKIP_ENERGY else NPANELS - 1
                    if SCHEME == "b":
                        if not (SKIP_ENERGY and p > 0):
                            for ns in range(NSUB):
                                qt_ps = pt.tile(
                                    [P, P], f32, tag="tp", name=f"qtps_{b}_{p}_{ns}"
                                )
                                for co in range(CCH):
                                    nc.tensor.matmul(
                                        qt_ps,
                                        x_cast(
                                            xf[:, co, ns * P : (ns + 1) * P], QK_DT
                                        ),
                                        wqk_sb[:, co, :],
                                        start=(co == 0),
                                        stop=(co == CCH - 1),
                                    )
                                qkt_sb = qktp.tile(
                                    [P, P],
                                    dt_of(EN_DT),
                                    tag="qkt",
                                    name=f"qkt_{b}_{p}_{ns}",
                                )
                                nc.vector.tensor_tensor(
                                    qkt_sb, qt_ps, bqk_bc, mybir.AluOpType.add
                                )
                                nc.tensor.matmul(
                                    energy,
                                    qkt_sb[:, 0:C8],
                                    qkt_sb[:, C8:P],
                                    start=(p == 0 and ns == 0),
                                    stop=(p == last_p and ns == NSUB - 1),
                                )
                    else:
                        qk_ps = pp.tile([P, NP], f32, tag="proj", name=f"qkps_{b}_{p}")
                        for co in range(CCH):
                            nc.tensor.matmul(
                                qk_ps,
                                wqk_sb[:, co, :],
                                x_cast(xf[:, co, :], QK_DT),
                                start=(co == 0),
                                stop=(co == CCH - 1),
                            )
                        qk_sb = qkp.tile(
                            [P, NP], dt_of(QK_DT), tag="qk", name=f"qk_{b}_{p}"
                        )
                        nc.scalar.activation(
                            qk_sb, qk_ps, Identity, bias=bqk_sb, scale=1.0
                        )
                        if not (SKIP_ENERGY and p > 0):
                            for ns in range(NSUB):
                                qdt = dt_of(QK_DT)
                                t_ps = pt.tile(
                                    [P, P], qdt, tag="tp", name=f"tps_{b}_{p}_{ns}"
                                )
                                nc.tensor.transpose(
                                    t_ps,
                                    qk_sb[:, ns * P : (ns + 1) * P],
                                    ident if qdt == f32 else ident_r,
                                )
                                qkt_sb = qktp.tile(
                                    [P, P],
                                    dt_of(EN_DT),
                                    tag="qkt",
                                    name=f"qkt_{b}_{p}_{ns}",
                                )
                                nc.vector.tensor_copy(qkt_sb, t_ps)
                                nc.tensor.matmul(
                                    energy,
                                    qkt_sb[:, 0:C8],
                                    qkt_sb[:, C8:P],
                                    start=(p == 0 and ns == 0),
                                    stop=(p == last_p and ns == NSUB - 1),
                                )

                # ---- Phase B: softmax, W2 = Wo @ (attn/rowsum), out = W2 @ v
                negmax = smallp.tile([C8, 1], f32, tag="negmax", name=f"negmax_{b}")
                nc.vector.reduce_max(
                    negmax, energy, axis=mybir.AxisListType.X, negate=True
                )
                attn = smallp.tile([C8, C8], f32, tag="attn", name=f"attn_{b}")
                rowsum = smallp.tile([C8, 1], f32, tag="rowsum", name=f"rowsum_{b}")
                nc.scalar.activation(
                    attn, energy, Exp, bias=negmax, scale=1.0, accum_out=rowsum
                )
                recip = smallp.tile([C8, 1], f32, tag="recip", name=f"recip_{b}")
                nc.vector.reciprocal(recip, rowsum)
                # normalize attn rows in place (per-partition scale)
                nc.vector.tensor_scalar_mul(attn, attn, recip)

                # W2T[d, o] = sum_c attn[c, d] WoT[c, o]  (one matmul)
                w2_ps = pt.tile([C8, C], f32, tag="tp", name=f"w2ps_{b}")
                nc.tensor.matmul(w2_ps, attn, wo_sb.bitcast(f32), start=True, stop=True)
                w2_sb = zp.tile([C8, C], dt_of(ZO_DT), tag="z", name=f"w2_{b}")
                nc.vector.tensor_copy(w2_sb, w2_ps)

                if SKIP_PHASEB:
                    assert Y_DT == "f32", "SKIP_PHASEB timing aid needs CHATT_Y_DT=f32"
                    for p in range(NPANELS):
                        nsl = slice(p * NP, (p + 1) * NP)
                        nc.sync.dma_start(yb[:C8, 0, nsl], v_sb[:, nsl].bitcast(f32))
                    continue
                if OUT_STAGE == "panel":
                    for p in range(NPANELS):
                        nsl = slice(p * NP, (p + 1) * NP)
                        o_sb = op.tile(
                            [P, CCH, NP], y_dt, tag="o", name=f"o_{b}_{p}"
                        )
                        for mo in range(CCH):
                            o_ps = pzo.tile(
                                [P, NP], f32, tag="zo", name=f"ops_{b}_{p}_{mo}"
                            )
                            nc.tensor.matmul(
                                o_ps,
                                w2_sb[:, mo * P : (mo + 1) * P],
                                v_sb[:, nsl],
                                start=True,
                                stop=True,
                            )
                            use_act = OUT_ENG == "act" or (
                                OUT_ENG == "alt" and mo % 2 == 1
                            )
                            if use_act:
                                nc.scalar.activation(
                                    o_sb[:, mo, :], o_ps, Identity,
                                    bias=bo_sb[:, mo : mo + 1], scale=1.0,
                                )
                            else:
                                nc.vector.tensor_scalar_add(
                                    o_sb[:, mo, :], o_ps, bo_sb[:, mo : mo + 1]
                                )
                        nc.sync.dma_start(yb[:, :, nsl], o_sb)
                else:
                    for mo in range(CCH):
                        for p in range(NPANELS):
                            nsl = slice(p * NP, (p + 1) * NP)
                            o_ps = pzo.tile(
                                [P, NP], f32, tag="zo", name=f"ops_{b}_{p}_{mo}"
                            )
                            nc.tensor.matmul(
                                o_ps,
                                w2_sb[:, mo * P : (mo + 1) * P],
                                v_sb[:, nsl],
                                start=True,
                                stop=True,
                            )
                            o_sb = op.tile(
                                [P, NP], y_dt, tag="o", name=f"o_{b}_{p}_{mo}"
                            )
                            use_act = OUT_ENG == "act" or (
                                OUT_ENG == "alt" and p % 2 == 1
                            )
                            if use_act:
                                nc.scalar.activation(
                                    o_sb, o_ps, Identity,
                                    bias=bo_sb[:, mo : mo + 1], scale=1.0,
                                )
                            else:
                                nc.vector.tensor_scalar_add(
                                    o_sb, o_ps, bo_sb[:, mo : mo + 1]
                                )
                            nc.sync.dma_start(yb[:, mo, nsl], o_sb)

    nc.compile()
    return nc


def _get_program():
    key = (QK_DT, V_DT, EN_DT, ZO_DT, Y_DT, SCHEME, REPS)
    if key not in _CACHE:
        _CACHE[key] = _build_program()
    return _CACHE[key]


def _host_inputs(x, Wq, bq, Wk, bk, Wv, bv, Wo, bo):
    """Build the per-core input maps (host-side shard + weight transposes)."""
    x = np.ascontiguousarray(x, dtype=np.float32).reshape(B, C, N)
    w_qkt = np.ascontiguousarray(
        np.concatenate([Wq, Wk], axis=0).T.astype(np.float32)
    )  # [C, 128]
    w_vt = np.ascontiguousarray(Wv.T.astype(np.float32))  # [C, 64]
    w_ot = np.ascontiguousarray(Wo.T.astype(np.float32))  # [64, C]
    b_qk = np.ascontiguousarray(
        np.concatenate([bq, bk], axis=0).astype(np.float32)
    )  # [128]
    b_v = np.ascontiguousarray(bv.astype(np.float32))
    b_o = np.ascontiguousarray(bo.astype(np.float32))

    in_maps = []
    for i in range(NCORES):
        in_maps.append(
            {
                "x": np.ascontiguousarray(x[i * BPC : (i + 1) * BPC]),
                "w_qkt": w_qkt,
                "w_vt": w_vt,
                "w_ot": w_ot,
                "b_qk": b_qk,
                "b_v": b_v,
                "b_o": b_o,
            }
        )
    return in_maps


def kernel(**inputs):
    global LAST_RESULTS
    from concourse.bass_utils import run_bass_kernel_spmd

    nc = _get_program()
    in_maps = _host_inputs(**inputs)
    res = run_bass_kernel_spmd(nc, in_maps, core_ids=list(range(NCORES)))
    LAST_RESULTS = res
    out = np.concatenate(
        [np.asarray(r["y"]).astype(np.float32) for r in res.results], axis=0
    )
    return out.reshape(B, C, H, W)

